# revision 1
# baseline (speedup 1.0000x reference)
"""Trainium2 Bass kernel for nn_Alignment_42236708389244.

Dense transformer block: 1x1conv+BN shortcut, cosine-normalized MHSA over L,
m-projection, then Linear -> LayerNorm -> exact GELU.

Sharding: data-parallel over batch (B=32 -> 4 per core x 8 cores). Each core
runs an identical program on its own batch slice; no collectives.

Layout strategy (per (b, t) sequence, C=512, L=512, H=8, D=64):
  - activations live feature-part ("X^T" = [C,L]) which is x's natural DRAM
    layout; projections use weights as lhsT (stationary) so no DMA transposes.
  - q/k/v are computed token-part ([L,C]) so the per-head l2-norm is a
    free-dim reduce; qn/kn are PE-transposed back to [D,L] for scores.
  - scores are computed TRANSPOSED (S^T = [l_k, l_q]); cosine scores are
    bounded (|s|<=1/8) so softmax needs no max-subtraction: E = exp(S^T),
    O^T = vn^T E / colsum(E) via matmuls (colsum via ones-lhsT, replicated).
  - LayerNorm over OU runs feature-part via (1/512)-ones matmul row sums.
  - ACT table sets: phase A uses only exp/ln (natural_log_exp set; rsqrt is
    exp(-.5*ln x)); GELU runs in a final phase over a DRAM-spilled z_pre so
    only two table loads happen.
Matmuls use float32r (full PE rate at N>=512, reduced-precision multiply).
"""

import sys

sys.path.insert(0, "/opt/trn_rl_repo")

import math
from contextlib import ExitStack

import numpy as np

import concourse.bacc as bacc
import concourse.bass as bass
import concourse.tile as tile
from concourse import mybir
from concourse.bass_utils import run_bass_kernel_spmd

B, C, T, L, H, D, OU = 32, 512, 4, 512, 8, 64, 512
NCORES = 8
BLOC = B // NCORES  # batches per core
NBT = BLOC * T  # (b, t) sequences per core
EPS = 1e-5
F32 = mybir.dt.float32
BF16 = mybir.dt.bfloat16
F16 = mybir.dt.float16
F32R = mybir.dt.float32r
AF = mybir.ActivationFunctionType
ALU = mybir.AluOpType

_cache = {}


def _r(ap):
    """View an fp32 AP as float32r for full-rate PE matmuls."""
    return ap.bitcast(F32R)


def _build(qkv_bias: bool = False):
    nc = bacc.Bacc(trn_type="TRN2", target_bir_lowering=False, debug=False)

    x_d = nc.dram_tensor("x", [BLOC, C, T, L], F32R, kind="ExternalInput")
    out_d = nc.dram_tensor("out", [BLOC, OU, T, L], F32, kind="ExternalOutput")
    w_names = ["wq", "wk", "wv", "wm", "wct", "wout"]
    w_d = {n: nc.dram_tensor(n, [C, C], F32R, kind="ExternalInput") for n in w_names}
    pet_d = nc.dram_tensor("pet", [C, L], F32R, kind="ExternalInput")
    qkvb_d = nc.dram_tensor("qkvb", [1, 3 * C], F32R, kind="ExternalInput")
    onesr_d = nc.dram_tensor("onesrd", [1, 128], F32R, kind="ExternalInput")
    # packed per-partition columns: [bias_sc, b_out, ln_g, ln_b] x 4 chunks
    cols_d = nc.dram_tensor("cols", [128, 16], F32, kind="ExternalInput")
    identbfd = nc.dram_tensor("identbfd", [128, 128], BF16, kind="ExternalInput")
    inv512d = nc.dram_tensor("inv512d", [128, 128], F32R, kind="ExternalInput")
    ones64d = nc.dram_tensor("ones64d", [128, 64], BF16, kind="ExternalInput")

    with tile.TileContext(nc) as tc, ExitStack() as ctx:
        con = ctx.enter_context(tc.tile_pool(name="con", bufs=1))
        wrk = ctx.enter_context(tc.tile_pool(name="wrk", bufs=1))
        pp = ctx.enter_context(tc.tile_pool(name="pp", bufs=1, space="PSUM"))
        drm = ctx.enter_context(tc.tile_pool(name="drm", bufs=1, space="DRAM"))
        z_scr = drm.tile([NBT, OU, L], F32, tag="zscr")

        # ---- constants ----
        ident_bf = con.tile([128, 128], BF16, tag="ident_bf")
        nc.sync.dma_start(out=ident_bf, in_=identbfd.ap())
        inv512 = con.tile([128, 128], F32R, tag="inv512")
        nc.sync.dma_start(out=inv512, in_=inv512d.ap())
        ones64 = con.tile([128, 64], BF16, tag="ones64")
        nc.sync.dma_start(out=ones64, in_=ones64d.ap())
        ln8_c = con.tile([128, 1], F32, tag="ln8_c")
        nc.vector.memset(ln8_c, math.log(0.125))
        eps_c = con.tile([128, 1], F32, tag="eps_c")
        nc.vector.memset(eps_c, EPS)
        cols = con.tile([128, 16], F32, tag="cols")
        nc.sync.dma_start(out=cols, in_=cols_d.ap())
        bias_sc = cols[:, 0:4]
        b_out_c = cols[:, 4:8]
        ln_g_c = cols[:, 8:12]
        ln_b_c = cols[:, 12:16]

        wt = {}
        for n in w_names:
            wt[n] = con.tile([128, 4, C], F32R, tag=n, name=f"wt_{n}")
            nc.sync.dma_start(
                out=wt[n], in_=w_d[n].ap().rearrange("(cc p) n -> p cc n", p=128)
            )
        qkvb = con.tile([1, 3 * C], F32R, tag="qkvb")
        nc.sync.dma_start(out=qkvb, in_=qkvb_d.ap())
        onesr = con.tile([1, 128], F32R, tag="onesr")
        nc.sync.dma_start(out=onesr, in_=onesr_d.ap())
        pet = con.tile([128, 4, L], F32R, tag="pet")
        nc.sync.dma_start(
            out=pet, in_=pet_d.ap().rearrange("(cc p) l -> p cc l", p=128)
        )

        for bt in range(NBT):
            b, t = bt // T, bt % T

            # ---- load X^T [c, l] as 4 chunks ----
            xt = wrk.tile([128, 4, L], F32R, tag="xt", bufs=2)
            nc.sync.dma_start(
                out=xt, in_=x_d.ap()[b, :, t, :].rearrange("(cc p) l -> p cc l", p=128)
            )

            # ---- xp = x + pe (feature-part), on gpsimd, per-chunk ----
            xp = wrk.tile([128, 4, L], F32R, tag="xp", bufs=2)
            for cc in range(4):
                nc.gpsimd.tensor_add(
                    out=xp[:, cc, :],
                    in0=xt.bitcast(F32)[:, cc, :],
                    in1=pet.bitcast(F32)[:, cc, :],
                )

            # ---- conv+BN shortcut into the out_sum accumulator tile ----
            os_t = wrk.tile([128, 4, L], F32R, tag="os", bufs=1, name="os_t")
            for oc in range(4):
                c_ps = pp.tile([128, 512], F32, tag="mm", bufs=2, name="c_ps")
                for cc in range(4):
                    nc.tensor.matmul(
                        c_ps,
                        wt["wct"][:, cc, oc * 128 : (oc + 1) * 128],
                        xt[:, cc, :],
                        start=(cc == 0),
                        stop=(cc == 3),
                    )
                nc.vector.tensor_scalar_add(
                    out=os_t[:, oc, :], in0=c_ps, scalar1=bias_sc[:, oc : oc + 1]
                )

            # ---- q/k/v projections (token-part [l, c]) + l2 norms ----
            qkv = {}
            for ti, (name, w) in enumerate(
                (("q", wt["wq"]), ("k", wt["wk"]), ("v", wt["wv"]))
            ):
                tn = wrk.tile(
                    [128, 4, C], BF16, tag=f"{name}n",
                    bufs=(2 if name == "v" else 1), name=f"{name}n",
                )
                for half in range(2):
                    p = pp.tile([128, 2 * C], F32, tag="proj", bufs=2, name="p")
                    for li in range(2):
                        lc = 2 * half + li
                        ps = p[:, li * C : (li + 1) * C]
                        if qkv_bias:
                            nc.tensor.matmul(
                                ps,
                                onesr[0:1, 0:128],
                                qkvb[0:1, ti * C : (ti + 1) * C],
                                start=True,
                                stop=False,
                            )
                        for cc in range(4):
                            nc.tensor.matmul(
                                ps,
                                xp[:, cc, lc * 128 : (lc + 1) * 128],
                                w[:, cc, :],
                                start=(cc == 0 and not qkv_bias),
                                stop=(cc == 3),
                            )
                    nc.scalar.copy(
                        out=tn[:, 2 * half : 2 * half + 2, :].rearrange(
                            "p lc n -> p (lc n)"
                        ),
                        in_=p,
                    )
                sq = wrk.tile([128, 4 * C], BF16, tag="sq", bufs=1, name="sq")
                tnf = tn.rearrange("p lc n -> p (lc n)")
                nc.vector.tensor_mul(out=sq, in0=tnf, in1=tnf)
                nrm = wrk.tile([128, 32], F32, tag="nrm", bufs=2, name="nrm")
                nc.vector.tensor_reduce(
                    nrm,
                    sq.rearrange("p (hh d) -> p hh d", d=D),
                    axis=mybir.AxisListType.X,
                    op=ALU.add,
                )
                # rsqrt via exp(-.5 * ln x); q also gets *1/8
                nc.scalar.activation(nrm, nrm, AF.Ln)
                nc.scalar.activation(
                    nrm, nrm, AF.Exp, bias=(ln8_c[:] if name == "q" else 0.0),
                    scale=-0.5,
                )
                sc_b = bass.AP(
                    tensor=nrm.tensor,
                    offset=nrm.offset,
                    ap=[list(nrm.ap[0]), list(nrm.ap[1]), [0, D]],
                )
                nc.vector.tensor_tensor(
                    tn.rearrange("p lc (h d) -> p (lc h) d", d=D),
                    tn.rearrange("p lc (h d) -> p (lc h) d", d=D),
                    sc_b,
                    ALU.mult,
                )
                qkv[name] = tn

            # ---- transpose qn, kn -> [c(part), l] ----
            tpt = {}
            for name in ("q", "k"):
                tn = qkv[name]
                tt = wrk.tile([128, 4, L], BF16, tag=f"{name}T", bufs=2, name=f"{name}T")
                for cc in range(4):
                    tp = pp.tile([128, 512], BF16, tag="mm", bufs=2, name="tp")
                    for lc in range(4):
                        nc.tensor.matmul(
                            tp[:, lc * 128 : (lc + 1) * 128],
                            tn[:, lc, cc * 128 : (cc + 1) * 128],
                            ident_bf,
                            is_transpose=True,
                            start=(lc == 0),
                            stop=(lc == 3),
                        )
                    nc.vector.tensor_copy(out=tt[:, cc, :], in_=tp)
                tpt[name] = tt
            qnT, knT = tpt["q"], tpt["k"]
            vn = qkv["v"]

            # ---- attention per head: S^T -> E -> O^T = vn^T E / colsum E ----
            oT = wrk.tile([128, 4, L], F32R, tag="oT", bufs=1, name="oT")
            for pair in range(4):
                o_ps = pp.tile([128, 512], F32, tag="mm", bufs=2, name="o_ps")
                zs_ps = pp.tile([128, 512], F32, tag="mm", bufs=2, name="zs_ps")
                for half in range(2):
                    h = 2 * pair + half
                    cc, r0 = h // 2, (h % 2) * 64
                    e = wrk.tile([128, 2048], BF16, tag="e", bufs=2, name="e")
                    for stp in range(2):
                        st = pp.tile([128, 1024], F32, tag="st", bufs=1, name="st")
                        for lq in range(2):
                            lk = stp * 2 + lq
                            nc.tensor.matmul(
                                st[:, lq * 512 : (lq + 1) * 512],
                                knT[r0 : r0 + 64, cc, lk * 128 : (lk + 1) * 128],
                                qnT[r0 : r0 + 64, cc, :],
                                start=True,
                                stop=True,
                            )
                        nc.scalar.activation(
                            e[:, stp * 1024 : (stp + 1) * 1024], st, AF.Exp
                        )
                    for lk in range(4):
                        nc.tensor.matmul(
                            o_ps[r0 : r0 + 64, :],
                            vn[:, lk, h * 64 : (h + 1) * 64],
                            e[:, lk * 512 : (lk + 1) * 512],
                            start=(lk == 0),
                            stop=(lk == 3),
                            tile_position=(0, r0),
                        )
                        nc.tensor.matmul(
                            zs_ps[r0 : r0 + 64, :],
                            ones64[:, 0:64],
                            e[:, lk * 512 : (lk + 1) * 512],
                            start=(lk == 0),
                            stop=(lk == 3),
                            tile_position=(0, r0),
                        )
                zr = wrk.tile([128, 512], F32, tag="zr", bufs=2, name="zr")
                nc.vector.reciprocal_approx_fast(out=zr, in_=zs_ps)
                nc.vector.tensor_mul(out=oT[:, pair, :], in0=o_ps, in1=zr)

            # ---- m-projection, accumulate onto shortcut in os_t ----
            for oc in range(4):
                m_ps = pp.tile([128, 512], F32, tag="mm", bufs=2, name="m_ps")
                for cc in range(4):
                    nc.tensor.matmul(
                        m_ps,
                        wt["wm"][:, cc, oc * 128 : (oc + 1) * 128],
                        oT[:, cc, :],
                        start=(cc == 0),
                        stop=(cc == 3),
                    )
                nc.vector.tensor_add(out=os_t[:, oc, :], in0=m_ps, in1=os_t.bitcast(F32)[:, oc, :])

            # ---- FFN out-projection z^T [ou, l] ----
            z_t = wrk.tile([128, 4, L], F32R, tag="zz", bufs=2, name="z_t")
            for oc in range(4):
                f_ps = pp.tile([128, 512], F32, tag="mm", bufs=2, name="f_ps")
                for cc in range(4):
                    nc.tensor.matmul(
                        f_ps,
                        wt["wout"][:, cc, oc * 128 : (oc + 1) * 128],
                        os_t[:, cc, :],
                        start=(cc == 0),
                        stop=(cc == 3),
                    )
                nc.vector.tensor_scalar_add(
                    out=z_t[:, oc, :], in0=f_ps, scalar1=b_out_c[:, oc : oc + 1]
                )

            # ---- LayerNorm over ou (partition dim) via ones-matmul row sums ----
            mu_ps = pp.tile([128, 512], F32, tag="mm", bufs=2, name="mu_ps")
            for oc in range(4):
                nc.tensor.matmul(
                    mu_ps, inv512, z_t[:, oc, :], start=(oc == 0),
                    stop=(oc == 3),
                )
            zc_t = wrk.tile([128, 4, L], F32, tag="zc", bufs=1, name="zc_t")
            sqz = wrk.tile([128, 4, L], F32R, tag="zz", bufs=2, name="sqz")
            for oc in range(4):
                nc.vector.tensor_sub(out=zc_t[:, oc, :], in0=z_t.bitcast(F32)[:, oc, :], in1=mu_ps)
                nc.scalar.activation(sqz[:, oc, :], zc_t[:, oc, :], AF.Square)
            var_ps = pp.tile([128, 512], F32, tag="mm", bufs=2, name="var_ps")
            for oc in range(4):
                nc.tensor.matmul(
                    var_ps, inv512, sqz[:, oc, :], start=(oc == 0),
                    stop=(oc == 3),
                )
            rstd = wrk.tile([128, 512], F32, tag="rstd", bufs=1, name="rstd")
            nc.scalar.activation(rstd, var_ps, AF.Ln, bias=eps_c[:])
            nc.scalar.activation(rstd, rstd, AF.Exp, scale=-0.5)
            for oc in range(4):
                nc.gpsimd.tensor_mul(out=zc_t[:, oc, :], in0=zc_t[:, oc, :], in1=rstd)
                nc.gpsimd.tensor_scalar(
                    out=zc_t[:, oc, :],
                    in0=zc_t[:, oc, :],
                    scalar1=ln_g_c[:, oc : oc + 1],
                    scalar2=ln_b_c[:, oc : oc + 1],
                    op0=ALU.mult,
                    op1=ALU.add,
                )
            nc.sync.dma_start(
                out=z_scr[bt, :, :].rearrange("(oc p) l -> p oc l", p=128),
                in_=zc_t,
            )

        # ---- phase B2: exact GELU epilogue (single gelu table load) ----
        for bt in range(NBT):
            b, t = bt // T, bt % T
            for oc in range(4):
                zin = wrk.tile([128, L], F32, tag="zin", bufs=8, name="zin")
                nc.sync.dma_start(
                    out=zin, in_=z_scr[bt, oc * 128 : (oc + 1) * 128, :]
                )
                tc.tile_set_cur_wait(10_000)  # gelu+store after phase-A ACT work
                nc.scalar.activation(zin, zin, AF.Gelu)
                nc.gpsimd.dma_start(
                    out=out_d.ap()[b, oc * 128 : (oc + 1) * 128, t, :], in_=zin
                )
                tc.tile_set_cur_wait(0)

    # Restrict ACT table-set choices so only 2 table loads are emitted:
    # natural_log_exp_and_others covers exp/ln/square/copy (phases A+B1),
    # gelu_and_others covers the B2 epilogue. Without this the insertion
    # pass alternates exp_and_others <-> natural_log per call (~130 loads).
    keep = {"natural_log_exp_and_others", "gelu_and_others"}
    orig_tables = bacc.get_activation_tables

    def patched_tables(arch):
        return {
            name: (funcs if name in keep else set())
            for name, funcs in orig_tables(arch).items()
        }

    bacc.get_activation_tables = patched_tables
    try:
        nc.finalize()
    finally:
        bacc.get_activation_tables = orig_tables
    return nc


def _prep(inputs):
    f = np.float32
    x = np.asarray(inputs["x"], f)
    pe = np.asarray(inputs["pe"], f)
    w_q, b_q = np.asarray(inputs["w_q"], f), np.asarray(inputs["b_q"], f)
    w_kv, b_kv = np.asarray(inputs["w_kv"], f), np.asarray(inputs["b_kv"], f)
    w_m, b_m = np.asarray(inputs["w_m"], f), np.asarray(inputs["b_m"], f)
    conv_w, conv_b = np.asarray(inputs["conv_w"], f), np.asarray(inputs["conv_b"], f)
    bn_g, bn_b = np.asarray(inputs["bn_gamma"], f), np.asarray(inputs["bn_beta"], f)
    bn_m, bn_v = np.asarray(inputs["bn_mean"], f), np.asarray(inputs["bn_var"], f)
    w_out, b_out = np.asarray(inputs["w_out"], f), np.asarray(inputs["b_out"], f)
    ln_g, ln_b = np.asarray(inputs["ln_g"], f), np.asarray(inputs["ln_b"], f)

    scale = bn_g / np.sqrt(bn_v + EPS)
    wct = np.ascontiguousarray((conv_w * scale[:, None]).T)  # [C, O]
    bias_sc = (conv_b - bn_m) * scale + bn_b + b_m  # b_m folded into shortcut

    w_k, w_v = w_kv[:, :C], w_kv[:, C:]
    import ml_dtypes

    bf = ml_dtypes.bfloat16
    pet = np.ascontiguousarray(pe.T)  # [C, L]

    def col(v):
        return np.ascontiguousarray(v.reshape(4, 128).T)  # [128, 4]

    cols = np.concatenate([col(bias_sc), col(b_out), col(ln_g), col(ln_b)], axis=1)

    shared = {
        "identbfd": np.eye(128, dtype=bf),
        "inv512d": np.full((128, 128), 1.0 / 512.0, dtype=f),
        "ones64d": np.ones((128, 64), dtype=bf),
        "wq": np.ascontiguousarray(w_q),
        "wk": np.ascontiguousarray(w_k),
        "wv": np.ascontiguousarray(w_v),
        "wm": np.ascontiguousarray(w_m),
        "wct": wct,
        "wout": np.ascontiguousarray(w_out),
        "pet": pet,
        "qkvb": np.concatenate([b_q, b_kv]).reshape(1, 3 * C).astype(f),
        "onesrd": np.ones((1, 128), dtype=f),
        "cols": np.ascontiguousarray(cols),
    }
    in_maps = []
    for core in range(NCORES):
        m = dict(shared)
        m["x"] = np.ascontiguousarray(x[core * BLOC : (core + 1) * BLOC])
        in_maps.append(m)
    return in_maps


def kernel(**inputs) -> np.ndarray:
    qb = bool(
        np.any(np.asarray(inputs["b_q"])) or np.any(np.asarray(inputs["b_kv"]))
    )
    key = ("nc", qb)
    if key not in _cache:
        _cache[key] = _build(qkv_bias=qb)
    nc = _cache[key]
    in_maps = _prep(inputs)
    res = run_bass_kernel_spmd(nc, in_maps, core_ids=list(range(NCORES)))
    return np.concatenate([r["out"] for r in res.results], axis=0)


if __name__ == "__main__":
    nc = _build()
    print("build ok; instructions:", len(nc.inst_map))

